# revision 42
# baseline (speedup 1.0000x reference)
"""AtomicNumberPooling Trainium2 kernel.

Math (from the reference):
    keys   = batch * 100 + (z - 1)                    # per-node (graph, bin) id
    sums   = segment_sum(out, keys, G * 100)          # [G*100, D]
    counts = nodes per graph                          # [G]
    pooled = sums.reshape(G, 100 * D) / max(counts, 1)

Strategy: data-parallel over graphs — 64 graphs per NeuronCore (8 cores).
x_rv_batch is sorted, so each graph's nodes are contiguous.  Per graph we
build a scaled one-hot matrix over the 100 atomic-number bins from the z
values (DVE tensor_scalar: is_equal(iota, z) * (1/count), computed in
fp32, stored bf16) and use TensorE to compute onehot.T @ features -> the
graph's [100, D] pooled block in PSUM (bf16 operands, fp32 accumulate).
PSUM banks hold 8 graphs' blocks side by side; ScalarE drains each bank
to SBUF and the result is DMA'd out bin-major ([100, C*D]) on alternating
HWDGE rings; the host transposes back to graph-major rows.

The SPMD program must be identical on all 8 cores, so every graph gets
exactly one 128-node "main" chunk; the rare graphs with >128 nodes get
overflow chunks appended after the 64 main chunks (same count C on every
core, padded with inert chunks whose one-hot is all-zero).  Overflow
partial sums land in extra output columns and are added into the right
graph row during the host-side gather.
"""

import numpy as np

NUM_Z = 100
G = 512
P = 128
NCORES = 8
GL = G // NCORES  # graphs per core
NA_F = 7  # leading chunks whose one-hot is built on ScalarE
NA_T = 3  # trailing chunks on ScalarE (built early, consumed last)

# filled by kernel() for optional inspection by a test harness
LAST_RESULTS = None


def _build_program(C, D):
    import concourse.bacc as bacc
    import concourse.mybir as mybir
    import concourse.tile as tile

    f32 = mybir.dt.float32
    bf16 = mybir.dt.bfloat16
    # Bacc (not plain Bass): its compile() legalizes sync waits — TRN2
    # instructions fit at most one wait, so multi-dep instructions need the
    # event-semaphore splitting pass.
    nc = bacc.Bacc("TRN2", debug=False, num_devices=NCORES)

    x_d = nc.dram_tensor("x", [P, C * D], bf16, kind="ExternalInput")
    # per-chunk (z, 1/count) pairs, then (-z, ln 1/count) pairs for the
    # chunks whose one-hots are built on ScalarE (exp(-30(iota-z)^2+ln s))
    act_chunks = list(range(NA_F)) + list(range(C - NA_T, C))
    act_idx = {j: t for t, j in enumerate(act_chunks)}
    cst_d = nc.dram_tensor(
        "cst", [P, 2 * C + 2 * len(act_chunks)], f32, kind="ExternalInput"
    )
    iota_d = nc.dram_tensor("iota", [P, NUM_Z], bf16, kind="ExternalInput")
    y_d = nc.dram_tensor("y", [NUM_Z, C * D], f32, kind="ExternalOutput")

    PER_BANK = 512 // D  # fp32 psum bank = 512 f32 -> 8 chunks of D=64
    BANK = PER_BANK * D
    NBANK = (C + PER_BANK - 1) // PER_BANK
    SGRP = 2             # banks per output store
    XHALF = (C + 1) // 2  # chunks per x load (2 loads total)

    with tile.TileContext(nc) as tc:
        with (
            tc.tile_pool(name="const", bufs=1) as constp,
            tc.tile_pool(name="xin", bufs=1) as xp,
            tc.tile_pool(name="oh", bufs=C) as ohp,
            tc.tile_pool(name="stage", bufs=3) as stp,
            tc.tile_pool(name="psum", bufs=8, space="PSUM") as pp,
        ):
            iota_t = constp.tile([P, NUM_Z], bf16)
            nc.sync.dma_start(iota_t[:], iota_d[:])
            cst_t = constp.tile([P, 2 * C + 2 * len(act_chunks)], f32)
            nc.sync.dma_start(cst_t[:], cst_d[:])

            def sc(q):  # f32 scalar view at column q
                return cst_t[:, q : q + 1]
            # two big feature loads; second overlaps first half's compute
            xa = xp.tile([P, XHALF * D], bf16, name="xa")
            nc.sync.dma_start(xa[:], x_d[:, : XHALF * D])
            xb = xp.tile([P, (C - XHALF) * D], bf16, name="xb")
            nc.sync.dma_start(xb[:], x_d[:, XHALF * D :])

            for b0 in range(0, NBANK, SGRP):
                bn = min(SGRP, NBANK - b0)
                cols0 = b0 * BANK
                colsn = min(C * D, (b0 + bn) * BANK) - cols0
                stage = stp.tile([P, SGRP * BANK], f32)
                for bb in range(bn):
                    s0 = (b0 + bb) * PER_BANK
                    sn = min(PER_BANK, C - s0)
                    ps = pp.tile([P, BANK], f32)
                    for jj in range(sn):
                        j = s0 + jj
                        oh = ohp.tile([P, NUM_Z], bf16)
                        if j in act_idx:
                            # ScalarE (idle until the first PSUM flush):
                            # oh = exp(-30*(iota - z)^2 + ln s) = s * [iota==z]
                            # (off-bin values <= s*1e-13, below bf16 noise)
                            t = 2 * C + 2 * act_idx[j]
                            sq = ohp.tile([P, NUM_Z], bf16, name=f"sq{j}")
                            nc.scalar.activation(
                                sq[:],
                                iota_t[:],
                                mybir.ActivationFunctionType.Square,
                                bias=sc(t),
                                scale=1.0,
                            )
                            nc.scalar.activation(
                                oh[:],
                                sq[:],
                                mybir.ActivationFunctionType.Exp,
                                bias=sc(t + 1),
                                scale=-30.0,
                            )
                        else:
                            nc.vector.tensor_scalar(
                                oh[:],
                                iota_t[:],
                                sc(2 * j),
                                sc(2 * j + 1),
                                mybir.AluOpType.is_equal,
                                mybir.AluOpType.mult,
                            )
                        if j < XHALF:
                            rhs = xa[:, j * D : (j + 1) * D]
                        else:
                            rhs = xb[:, (j - XHALF) * D : (j - XHALF + 1) * D]
                        nc.tensor.matmul(
                            out=ps[:NUM_Z, jj * D : (jj + 1) * D],
                            lhsT=oh[:],
                            rhs=rhs,
                            start=True,
                            stop=True,
                        )
                    # drain bank -> stage (scale is already in the one-hot).
                    # Second-to-last bank drains on DVE (idle by then), the
                    # tiny last bank on ScalarE — the two run in parallel
                    if b0 + bb == NBANK - 2:
                        nc.vector.tensor_copy(
                            stage[:NUM_Z, bb * BANK : bb * BANK + sn * D],
                            ps[:NUM_Z, : sn * D],
                        )
                    else:
                        nc.scalar.copy(
                            stage[:NUM_Z, bb * BANK : bb * BANK + sn * D],
                            ps[:NUM_Z, : sn * D],
                        )
                # store SGRP banks per DMA: sync ring (idle after loads),
                # except the last group on the scalar ring so the two final
                # store triggers don't serialize
                seng = nc.scalar if b0 + SGRP >= NBANK else nc.sync
                seng.dma_start(
                    y_d[:, cols0 : cols0 + colsn], stage[:NUM_Z, :colsn]
                )
    nc.compile()
    return nc


def _prep(x, z, b, D):
    """Build per-core padded inputs.  Returns (in_maps, over_maps, C)."""
    import ml_dtypes

    counts = np.bincount(b, minlength=G).astype(np.int64)
    starts = np.zeros(G + 1, np.int64)
    np.cumsum(counts, out=starts[1:])

    per_core = []
    for k in range(NCORES):
        main = []  # (node_start, length, graph) — one per graph, in order
        over = []  # extra pieces for graphs with >P nodes
        for gl in range(GL):
            g = k * GL + gl
            s, n = int(starts[g]), int(counts[g])
            main.append((s, min(n, P), g))
            off = P
            while off < n:
                over.append((s + off, min(n - off, P), g))
                off += P
        per_core.append((main, over))

    B = max(len(o) for _, o in per_core)
    C = GL + B

    in_maps, over_maps = [], []
    for k in range(NCORES):
        main, over = per_core[k]
        chunks = main + over
        act_chunks = list(range(NA_F)) + list(range(C - NA_T, C))
        act_idx = {j: t for t, j in enumerate(act_chunks)}
        xT = np.zeros((P, C, D), ml_dtypes.bfloat16)
        cst = np.zeros((P, 2 * C + 2 * len(act_chunks)), np.float32)
        cst[:, 0 : 2 * C : 2] = -1.0  # pad z: never matches iota 0..99
        cst[:, 2 * C :: 2] = 1.0  # pad -z for ScalarE chunks (filled below)
        for j, (s, ln, g) in enumerate(chunks):
            xT[:ln, j, :] = x[s : s + ln].astype(ml_dtypes.bfloat16)
            cst[:ln, 2 * j] = z[s : s + ln]
            cst[:, 2 * j + 1] = 1.0 / max(int(counts[g]), 1)
            if j in act_idx:
                t = 2 * C + 2 * act_idx[j]
                cst[:ln, t] = -z[s : s + ln].astype(np.float32)
                cst[:, t + 1] = -np.log(max(int(counts[g]), 1))
        iota = np.broadcast_to(
            np.arange(NUM_Z, dtype=ml_dtypes.bfloat16), (P, NUM_Z)
        ).copy()
        in_maps.append(
            {
                "x": np.ascontiguousarray(xT.reshape(P, C * D)),
                "cst": cst,
                "iota": iota,
            }
        )
        over_maps.append([(GL + j, g) for j, (s, ln, g) in enumerate(over)])
    return in_maps, over_maps, C


def _ensure_ntff_hook():
    """run_bass_kernel_spmd(trace=True) under axon imports antenv.axon_hooks,
    which this agent image lacks — recreate it (with the ctypes NTFF hook if
    available) so a BASS_TRACE=1 environment doesn't crash kernel()."""
    import sys
    import types

    try:
        import antenv.axon_hooks  # noqa: F401

        return
    except ImportError:
        pass
    try:
        import antenv
    except ImportError:
        return
    hook = None
    try:
        from trn_agent_boot.trn_boot import _ntff_profile_via_ctypes

        hook = _ntff_profile_via_ctypes("/opt/axon/libaxon_pjrt.so")
    except Exception:
        pass
    mod = types.ModuleType("antenv.axon_hooks")
    mod._hook = hook
    mod.get_axon_ntff_profile_hook = lambda: mod._hook
    mod.set_axon_ntff_profile_hook = lambda h: setattr(mod, "_hook", h)
    sys.modules["antenv.axon_hooks"] = mod
    antenv.axon_hooks = mod


def kernel(out, z_rv, x_rv_batch):
    global LAST_RESULTS
    from concourse.bass_utils import run_bass_kernel_spmd

    _ensure_ntff_hook()

    x = np.ascontiguousarray(np.asarray(out), dtype=np.float32)
    z = np.asarray(z_rv).astype(np.int64) - 1  # 0..99
    b = np.asarray(x_rv_batch).astype(np.int64)
    D = x.shape[1]

    in_maps, over_maps, C = _prep(x, z, b, D)
    nc = _build_program(C, D)
    res = run_bass_kernel_spmd(nc, in_maps, core_ids=list(range(NCORES)))
    LAST_RESULTS = res

    full = np.empty((G, NUM_Z * D), np.float32)
    for k in range(NCORES):
        yk = np.asarray(res.results[k]["y"])  # [NUM_Z, C*D]
        blocks = (
            yk.reshape(NUM_Z, C, D).transpose(1, 0, 2).reshape(C, NUM_Z * D)
        )
        full[k * GL : (k + 1) * GL] = blocks[:GL]
        for j, g in over_maps[k]:
            full[g] += blocks[j]
    return full


# revision 45
# speedup vs baseline: 1.0061x; 1.0061x over previous
"""AtomicNumberPooling Trainium2 kernel.

Math (from the reference):
    keys   = batch * 100 + (z - 1)                    # per-node (graph, bin) id
    sums   = segment_sum(out, keys, G * 100)          # [G*100, D]
    counts = nodes per graph                          # [G]
    pooled = sums.reshape(G, 100 * D) / max(counts, 1)

Strategy: data-parallel over graphs — 64 graphs per NeuronCore (8 cores).
x_rv_batch is sorted, so each graph's nodes are contiguous.  Per graph we
build a scaled one-hot matrix over the 100 atomic-number bins from the z
values — mostly on VectorE (tensor_scalar: is_equal(iota, z) * (1/count),
fp32 math, bf16 out), with a few chunks on the otherwise-idle ScalarE via
exp(-30*(iota-z)^2 + ln s) — and use TensorE to compute onehot.T @
features -> the graph's [100, D] pooled block in PSUM (bf16 operands,
fp32 accumulate).  PSUM banks hold 8 graphs' blocks side by side; ScalarE
drains each bank to SBUF (VectorE takes the second-to-last so the tail
parallelizes) and results stream out bin-major ([100, C*D]) mostly on the
sync HWDGE ring; the host transposes back to graph-major rows.

The SPMD program must be identical on all 8 cores, so every graph gets
exactly one 128-node "main" chunk; the rare graphs with >128 nodes get
overflow chunks appended after the 64 main chunks (same count C on every
core, padded with inert chunks whose one-hot is all-zero).  Overflow
partial sums land in extra output columns and are added into the right
graph row during the host-side gather.
"""

import numpy as np

NUM_Z = 100
G = 512
P = 128
NCORES = 8
GL = G // NCORES  # graphs per core
NA_F = 7  # leading chunks whose one-hot is built on ScalarE
NA_T = 3  # trailing chunks on ScalarE (built early, consumed last)

# filled by kernel() for optional inspection by a test harness
LAST_RESULTS = None


def _build_program(C, D):
    import concourse.bacc as bacc
    import concourse.mybir as mybir
    import concourse.tile as tile

    f32 = mybir.dt.float32
    bf16 = mybir.dt.bfloat16
    # Bacc (not plain Bass): its compile() legalizes sync waits — TRN2
    # instructions fit at most one wait, so multi-dep instructions need the
    # event-semaphore splitting pass.
    nc = bacc.Bacc("TRN2", debug=False, num_devices=NCORES)

    x_d = nc.dram_tensor("x", [P, C * D], bf16, kind="ExternalInput")
    # per-chunk (z, 1/count) pairs, then (-z, ln 1/count) pairs for the
    # chunks whose one-hots are built on ScalarE (exp(-30(iota-z)^2+ln s))
    act_chunks = list(range(NA_F)) + list(range(C - NA_T, C))
    act_idx = {j: t for t, j in enumerate(act_chunks)}
    cst_d = nc.dram_tensor(
        "cst", [P, 2 * C + 2 * len(act_chunks)], f32, kind="ExternalInput"
    )
    iota_d = nc.dram_tensor("iota", [P, NUM_Z], bf16, kind="ExternalInput")
    y_d = nc.dram_tensor("y", [NUM_Z, C * D], f32, kind="ExternalOutput")

    PER_BANK = 512 // D  # fp32 psum bank = 512 f32 -> 8 chunks of D=64
    BANK = PER_BANK * D
    NBANK = (C + PER_BANK - 1) // PER_BANK
    SGRP = 2             # banks per output store
    XHALF = (C + 1) // 2  # chunks per x load (2 loads total)

    with tile.TileContext(nc) as tc:
        with (
            tc.tile_pool(name="const", bufs=1) as constp,
            tc.tile_pool(name="xin", bufs=1) as xp,
            tc.tile_pool(name="oh", bufs=C) as ohp,
            tc.tile_pool(name="stage", bufs=3) as stp,
            tc.tile_pool(name="psum", bufs=8, space="PSUM") as pp,
        ):
            iota_t = constp.tile([P, NUM_Z], bf16)
            nc.sync.dma_start(iota_t[:], iota_d[:])
            cst_t = constp.tile([P, 2 * C + 2 * len(act_chunks)], f32)
            nc.sync.dma_start(cst_t[:], cst_d[:])

            def sc(q):  # f32 scalar view at column q
                return cst_t[:, q : q + 1]
            # two big feature loads; second overlaps first half's compute
            xa = xp.tile([P, XHALF * D], bf16, name="xa")
            nc.sync.dma_start(xa[:], x_d[:, : XHALF * D])
            xb = xp.tile([P, (C - XHALF) * D], bf16, name="xb")
            nc.sync.dma_start(xb[:], x_d[:, XHALF * D :])

            for b0 in range(0, NBANK, SGRP):
                bn = min(SGRP, NBANK - b0)
                cols0 = b0 * BANK
                colsn = min(C * D, (b0 + bn) * BANK) - cols0
                stage = stp.tile([P, SGRP * BANK], f32)
                for bb in range(bn):
                    s0 = (b0 + bb) * PER_BANK
                    sn = min(PER_BANK, C - s0)
                    ps = pp.tile([P, BANK], f32)
                    for jj in range(sn):
                        j = s0 + jj
                        oh = ohp.tile([P, NUM_Z], bf16)
                        if j in act_idx:
                            # ScalarE (idle until the first PSUM flush):
                            # oh = exp(-30*(iota - z)^2 + ln s) = s * [iota==z]
                            # (off-bin values <= s*1e-13, below bf16 noise)
                            t = 2 * C + 2 * act_idx[j]
                            sq = ohp.tile([P, NUM_Z], bf16, name=f"sq{j}")
                            nc.scalar.activation(
                                sq[:],
                                iota_t[:],
                                mybir.ActivationFunctionType.Square,
                                bias=sc(t),
                                scale=1.0,
                            )
                            nc.scalar.activation(
                                oh[:],
                                sq[:],
                                mybir.ActivationFunctionType.Exp,
                                bias=sc(t + 1),
                                scale=-30.0,
                            )
                        else:
                            nc.vector.tensor_scalar(
                                oh[:],
                                iota_t[:],
                                sc(2 * j),
                                sc(2 * j + 1),
                                mybir.AluOpType.is_equal,
                                mybir.AluOpType.mult,
                            )
                        if j < XHALF:
                            rhs = xa[:, j * D : (j + 1) * D]
                        else:
                            rhs = xb[:, (j - XHALF) * D : (j - XHALF + 1) * D]
                        nc.tensor.matmul(
                            out=ps[:NUM_Z, jj * D : (jj + 1) * D],
                            lhsT=oh[:],
                            rhs=rhs,
                            start=True,
                            stop=True,
                        )
                    # drain bank -> stage (scale is already in the one-hot).
                    # Second-to-last bank drains on DVE (idle by then), the
                    # tiny last bank on ScalarE — the two run in parallel
                    if b0 + bb == NBANK - 2:
                        nc.vector.tensor_copy(
                            stage[:NUM_Z, bb * BANK : bb * BANK + sn * D],
                            ps[:NUM_Z, : sn * D],
                        )
                    else:
                        nc.scalar.copy(
                            stage[:NUM_Z, bb * BANK : bb * BANK + sn * D],
                            ps[:NUM_Z, : sn * D],
                        )
                # store SGRP banks per DMA: sync ring (idle after loads),
                # except the last group on the scalar ring so the two final
                # store triggers don't serialize
                seng = nc.scalar if b0 + SGRP >= NBANK else nc.sync
                seng.dma_start(
                    y_d[:, cols0 : cols0 + colsn], stage[:NUM_Z, :colsn]
                )
    nc.compile()
    return nc


def _prep(x, z, b, D):
    """Build per-core padded inputs.  Returns (in_maps, over_maps, C)."""
    import ml_dtypes

    counts = np.bincount(b, minlength=G).astype(np.int64)
    starts = np.zeros(G + 1, np.int64)
    np.cumsum(counts, out=starts[1:])

    per_core = []
    for k in range(NCORES):
        main = []  # (node_start, length, graph) — one per graph, in order
        over = []  # extra pieces for graphs with >P nodes
        for gl in range(GL):
            g = k * GL + gl
            s, n = int(starts[g]), int(counts[g])
            main.append((s, min(n, P), g))
            off = P
            while off < n:
                over.append((s + off, min(n - off, P), g))
                off += P
        per_core.append((main, over))

    B = max(len(o) for _, o in per_core)
    C = GL + B

    in_maps, over_maps = [], []
    for k in range(NCORES):
        main, over = per_core[k]
        chunks = main + over
        act_chunks = list(range(NA_F)) + list(range(C - NA_T, C))
        act_idx = {j: t for t, j in enumerate(act_chunks)}
        xT = np.zeros((P, C, D), ml_dtypes.bfloat16)
        cst = np.zeros((P, 2 * C + 2 * len(act_chunks)), np.float32)
        cst[:, 0 : 2 * C : 2] = -1.0  # pad z: never matches iota 0..99
        cst[:, 2 * C :: 2] = 1.0  # pad -z for ScalarE chunks (filled below)
        for j, (s, ln, g) in enumerate(chunks):
            xT[:ln, j, :] = x[s : s + ln].astype(ml_dtypes.bfloat16)
            cst[:ln, 2 * j] = z[s : s + ln]
            cst[:, 2 * j + 1] = 1.0 / max(int(counts[g]), 1)
            if j in act_idx:
                t = 2 * C + 2 * act_idx[j]
                cst[:ln, t] = -z[s : s + ln].astype(np.float32)
                cst[:, t + 1] = -np.log(max(int(counts[g]), 1))
        iota = np.broadcast_to(
            np.arange(NUM_Z, dtype=ml_dtypes.bfloat16), (P, NUM_Z)
        ).copy()
        in_maps.append(
            {
                "x": np.ascontiguousarray(xT.reshape(P, C * D)),
                "cst": cst,
                "iota": iota,
            }
        )
        over_maps.append([(GL + j, g) for j, (s, ln, g) in enumerate(over)])
    return in_maps, over_maps, C


def _ensure_ntff_hook():
    """run_bass_kernel_spmd(trace=True) under axon imports antenv.axon_hooks,
    which this agent image lacks — recreate it (with the ctypes NTFF hook if
    available) so a BASS_TRACE=1 environment doesn't crash kernel()."""
    import sys
    import types

    try:
        import antenv.axon_hooks  # noqa: F401

        return
    except ImportError:
        pass
    try:
        import antenv
    except ImportError:
        return
    hook = None
    try:
        from trn_agent_boot.trn_boot import _ntff_profile_via_ctypes

        hook = _ntff_profile_via_ctypes("/opt/axon/libaxon_pjrt.so")
    except Exception:
        pass
    mod = types.ModuleType("antenv.axon_hooks")
    mod._hook = hook
    mod.get_axon_ntff_profile_hook = lambda: mod._hook
    mod.set_axon_ntff_profile_hook = lambda h: setattr(mod, "_hook", h)
    sys.modules["antenv.axon_hooks"] = mod
    antenv.axon_hooks = mod


def kernel(out, z_rv, x_rv_batch):
    global LAST_RESULTS
    from concourse.bass_utils import run_bass_kernel_spmd

    _ensure_ntff_hook()

    x = np.ascontiguousarray(np.asarray(out), dtype=np.float32)
    z = np.asarray(z_rv).astype(np.int64) - 1  # 0..99
    b = np.asarray(x_rv_batch).astype(np.int64)
    D = x.shape[1]

    in_maps, over_maps, C = _prep(x, z, b, D)
    nc = _build_program(C, D)
    res = run_bass_kernel_spmd(nc, in_maps, core_ids=list(range(NCORES)))
    LAST_RESULTS = res

    full = np.empty((G, NUM_Z * D), np.float32)
    for k in range(NCORES):
        yk = np.asarray(res.results[k]["y"])  # [NUM_Z, C*D]
        blocks = (
            yk.reshape(NUM_Z, C, D).transpose(1, 0, 2).reshape(C, NUM_Z * D)
        )
        full[k * GL : (k + 1) * GL] = blocks[:GL]
        for j, g in over_maps[k]:
            full[g] += blocks[j]
    return full
